# revision 1
# baseline (speedup 1.0000x reference)
"""Multi-head attention (B=4, N=2048, C=768, H=12, D=64) on 8 TRN2 NeuronCores.

Sharding: core c handles batch b=c//2 and a half of the heads (6 heads,
g=c%2).  Each core computes q/k/v projections for its head slice, S^T-layout
attention (scores transposed: nk on partitions, nq on free), softmax without
max-subtraction (scores are ~N(0,1); exp is safe in fp32), PV with V as the
stationary operand producing out^T (bf16, col-tiled head pairs), per-head 1/Z
scaling folded into the PSUM eviction, and a partial output projection.  Host
sums the two per-batch partials and adds bo.  bk is ignored (softmax
shift-invariant); bq folded into the qT eviction; bv folded into the v
eviction.

Matmuls run in float32r (rounded fp32, full PE rate; ~1.6e-4 max-rel per
matmul measured on HW) except PV which is bf16 (col-tiling is not supported
by codegen for f32r).  Z is accumulated two-level: bf16 minor sums over
groups of 4 nk-tiles (DVE 4x mode) + f32 major adds, then reduced across
partitions with a ones-vector matmul and inverted on DVE.

Emission order is tuned so the scalar engine (exp) is the critical path:
pair-0 q/k projections first, then v, then attention; pair p+1's projections
are emitted right after attn(p, ch0) so they fill PE slack under the
ACT-bound attention pipeline.
"""

import numpy as np

B, N, C = 4, 2048, 768
H, D = 12, 64
HPC = 6                 # heads per core
DV = HPC * D            # 384
P = 128
KC = C // P             # 6 contraction chunks for projections
NPAIR = DV // P         # 3 head-pairs per core
NT = N // P             # 16 nk tiles
SEG = 512
CHUNK = 1024
NSEG_CH = CHUNK // SEG  # 2 segments per chunk
NCH = N // CHUNK        # 2 chunks
SCALE = 1.0 / np.sqrt(D)

_CACHE = {}


def _build(reps=1, noz=False):
    import warnings
    warnings.filterwarnings("ignore")
    import concourse.bass as bass
    import concourse.bacc as bacc
    import concourse.mybir as mybir
    from concourse import tile

    f32 = mybir.dt.float32
    f32r = mybir.dt.float32r
    bf16 = mybir.dt.bfloat16
    Act = mybir.ActivationFunctionType

    nc = bacc.Bacc("TRN2", target_bir_lowering=False, debug=False)

    xT = nc.dram_tensor("xT", [C, N], f32r, kind="ExternalInput").ap()
    wqT = nc.dram_tensor("wqT", [C, DV], f32r, kind="ExternalInput").ap()
    wkT = nc.dram_tensor("wkT", [C, DV], f32r, kind="ExternalInput").ap()
    wvT = nc.dram_tensor("wvT", [C, DV], f32r, kind="ExternalInput").ap()
    woT = nc.dram_tensor("woT", [DV, C], f32r, kind="ExternalInput").ap()
    bq = nc.dram_tensor("bq", [DV, 1], f32, kind="ExternalInput").ap()
    bv = nc.dram_tensor("bv", [1, DV], f32, kind="ExternalInput").ap()
    ones = nc.dram_tensor("ones", [P, 1], f32r, kind="ExternalInput").ap()
    y = nc.dram_tensor("y", [N, C], f32, kind="ExternalOutput").ap()

    with tile.TileContext(nc) as tc:
        import contextlib
        with contextlib.ExitStack() as ctx:
            ec = ctx.enter_context
            p_xt = ec(tc.tile_pool(name="p_xt", bufs=KC))
            p_qk = ec(tc.tile_pool(name="p_qk", bufs=2 * NPAIR))
            p_v = ec(tc.tile_pool(name="p_v", bufs=NT))
            p_wqk = ec(tc.tile_pool(name="p_wqk", bufs=3))
            p_wv = ec(tc.tile_pool(name="p_wv", bufs=KC))
            p_wo = ec(tc.tile_pool(name="p_wo", bufs=NPAIR))
            p_exp = ec(tc.tile_pool(name="p_exp", bufs=6))
            p_zm = ec(tc.tile_pool(name="p_zm", bufs=3))
            p_at = ec(tc.tile_pool(name="p_at", bufs=5))
            p_z = ec(tc.tile_pool(name="p_z", bufs=2))
            p_rz = ec(tc.tile_pool(name="p_rz", bufs=2))
            p_rzrow = ec(tc.tile_pool(name="p_rzrow", bufs=2))
            p_ysb = ec(tc.tile_pool(name="p_ysb", bufs=2))
            p_small = ec(tc.tile_pool(name="p_small", bufs=1))
            p_dram = ec(tc.tile_pool(name="p_dram", bufs=4, space="DRAM"))
            # PSUM: stage 2x2 banks + pv 2 banks + aux 2x1 banks = 8
            p_stage = ec(tc.tile_pool(name="p_stage", bufs=2, space="PSUM"))
            p_pv = ec(tc.tile_pool(name="p_pv", bufs=1, space="PSUM"))
            p_aux = ec(tc.tile_pool(name="p_aux", bufs=2, space="PSUM"))

            for _rep in range(reps):
                # ---- big x input: split DMAs over both HWDGE queues ----
                xt_sb = [p_xt.tile([P, N], f32r, tag="xt", name=f"xt{k}")
                         for k in range(KC)]

                def load_xt():
                    for hlf in range(2):
                        for k in range(KC):
                            eng = nc.sync if k % 2 == 0 else nc.scalar
                            eng.dma_start(
                                xt_sb[k][:, hlf * CHUNK:(hlf + 1) * CHUNK],
                                xT[k * P:(k + 1) * P, hlf * CHUNK:(hlf + 1) * CHUNK])

                # ---- constant/small loads ----
                ones_sb = p_small.tile([P, 1], f32r, name="ones_sb")
                nc.scalar.dma_start(ones_sb[:], ones[:, :])
                bq_sb = [p_small.tile([P, 1], f32, name=f"bq{i}") for i in range(NPAIR)]
                for i in range(NPAIR):
                    nc.sync.dma_start(bq_sb[i][:], bq[i * P:(i + 1) * P, :])
                bv_bc = p_small.tile([P, DV], f32, name="bv_bc")
                bv_bcast_ap = bass.AP(tensor=bv.tensor, offset=0,
                                      ap=[[0, P]] + [list(a) for a in bv.ap[1:]])
                nc.gpsimd.dma_start(bv_bc[:], bv_bcast_ap)

                def load_w_pair(src, p, label):
                    # [C, DV] column block for pair p -> [128, KC, 128] in one DMA
                    t = p_wqk.tile([P, KC, P], f32r, tag="wqk", name=f"{label}{p}")
                    blk = src[:, p * P:(p + 1) * P].rearrange(
                        "(k r) m -> r k m", r=P)
                    nc.sync.dma_start(t[:], blk)
                    return t

                def proj_qk_seg(p, wq_sb, wk_sb, qT, kT, j):
                    if True:
                        q_ps = p_aux.tile([P, SEG], f32, tag="aux", name=f"qps{p}_{j}")
                        for k in range(KC):
                            nc.tensor.matmul(
                                q_ps[:], wq_sb[:, k, :],
                                xt_sb[k][:, j * SEG:(j + 1) * SEG],
                                start=(k == 0), stop=(k == KC - 1))
                        nc.vector.tensor_scalar_add(
                            qT[:, j * SEG:(j + 1) * SEG], q_ps[:], bq_sb[p][:])
                        k_ps = p_aux.tile([P, SEG], f32, tag="aux", name=f"kps{p}_{j}")
                        for k in range(KC):
                            nc.tensor.matmul(
                                k_ps[:], wk_sb[:, k, :],
                                xt_sb[k][:, j * SEG:(j + 1) * SEG],
                                start=(k == 0), stop=(k == KC - 1))
                        nc.vector.tensor_copy(kT[:, j * SEG:(j + 1) * SEG], k_ps[:])

                qT = [None] * NPAIR
                kT = [None] * NPAIR

                wpair = [None] * NPAIR

                def prep_proj(p):
                    wpair[p] = (load_w_pair(wqT, p, "wq"),
                                load_w_pair(wkT, p, "wk"))
                    qT[p] = p_qk.tile([P, N], f32r, tag="qk", name=f"qT{p}")
                    kT[p] = p_qk.tile([P, N], f32r, tag="qk", name=f"kT{p}")

                def emit_proj(p, js=None):
                    if qT[p] is None:
                        prep_proj(p)
                    wq_sb, wk_sb = wpair[p]
                    for j in (range(N // SEG) if js is None else js):
                        proj_qk_seg(p, wq_sb, wk_sb, qT[p], kT[p], j)

                # pair-0 weight DMAs go on the sync queue ahead of the big xT
                # transfers; projection matmuls are emitted after load_xt so
                # Tile's trace-order dependencies see DMA-before-read
                prep_proj(0)
                load_xt()
                emit_proj(0)

                # ---- v projection: v_sb[t][seq 128, dv 384], all heads ----
                wv_sb = [p_wv.tile([P, DV], f32r, tag="wv", name=f"wv{k}")
                         for k in range(KC)]
                for k in range(KC):
                    nc.scalar.dma_start(wv_sb[k][:], wvT[k * P:(k + 1) * P, :])
                wo_sb = [p_wo.tile([P, C], f32r, tag="wo", name=f"wo{p}")
                         for p in range(NPAIR)]
                for p in range(NPAIR):
                    nc.scalar.dma_start(wo_sb[p][:], woT[p * P:(p + 1) * P, :])
                v_sb = [p_v.tile([P, DV], bf16, tag="v", name=f"v{t}")
                        for t in range(NT)]

                def emit_vproj(ts):
                    for t in ts:
                        v_ps = p_aux.tile([P, DV], f32, tag="aux", name=f"vps{t}")
                        for k in range(KC):
                            nc.tensor.matmul(
                                v_ps[:], xt_sb[k][:, t * P:(t + 1) * P], wv_sb[k][:],
                                start=(k == 0), stop=(k == KC - 1))
                        nc.vector.tensor_add(v_sb[t][:], v_ps[:], bv_bc[:])

                emit_vproj(range(NT))

                aT = [[None] * NCH for _ in range(NPAIR)]

                def emit_attn(p, ch):
                    q0 = ch * CHUNK
                    # two-level Z: bf16 minor sums (DVE 4x) over groups of 4
                    # nk-tiles, f32 major accumulation every 4th tile
                    z_acc = ([p_z.tile([P, CHUNK], f32r, tag="z",
                                       name=f"z{p}_{ch}_{h}") for h in range(2)]
                             if not noz else [None, None])
                    z_min = [None, None]
                    elast = [[], []]
                    ztree = [[], []]
                    pv = p_pv.tile([P, CHUNK], f32, tag="pv", name=f"pv{p}_{ch}")
                    for t in range(NT):
                        st = [p_stage.tile([P, CHUNK], f32, tag="st",
                                           name=f"st{p}_{ch}_{t}_{h}")
                              for h in range(2)]
                        for h in range(2):
                            hp = h * 64
                            for sg in range(NSEG_CH):
                                nc.tensor.matmul(
                                    st[h][:, sg * SEG:(sg + 1) * SEG],
                                    kT[p][hp:hp + 64, t * P:(t + 1) * P],
                                    qT[p][hp:hp + 64,
                                          q0 + sg * SEG:q0 + (sg + 1) * SEG],
                                    start=True, stop=True,
                                    tile_position=(hp, 0))
                        e = [p_exp.tile([P, CHUNK], bf16, tag="e",
                                        name=f"e{p}_{ch}_{t}_{h}")
                             for h in range(2)]
                        for h in range(2):
                            nc.scalar.activation(e[h][:], st[h][:], Act.Exp,
                                                 scale=float(SCALE))
                            if not noz:
                                # Z accumulation: independent pair-adds of
                                # consecutive exp tiles, then a half-rate
                                # serial chain into z_acc.  Halves the chain
                                # depth vs per-t accumulation so DVE doesn't
                                # pace the exp pipeline.
                                elast[h].append(e[h])
                                if len(elast[h]) == 2:
                                    ea, eb = elast[h]
                                    elast[h] = []
                                    if t == 1:
                                        nc.vector.tensor_add(
                                            z_acc[h][:], ea[:], eb[:])
                                    else:
                                        zp = p_zm.tile([P, CHUNK], bf16,
                                                       tag="zm",
                                                       name=f"zp{p}_{ch}_{t}_{h}")
                                        nc.vector.tensor_add(zp[:], ea[:], eb[:])
                                        nc.vector.tensor_add(
                                            z_acc[h][:],
                                            z_acc[h][:].bitcast(f32), zp[:])
                            hp = h * 64
                            for sg in range(NSEG_CH):
                                nc.tensor.matmul(
                                    pv[hp:hp + 64, sg * SEG:(sg + 1) * SEG],
                                    v_sb[t][:, p * P + hp:p * P + hp + 64],
                                    e[h][:, sg * SEG:(sg + 1) * SEG],
                                    start=(t == 0), stop=(t == NT - 1),
                                    tile_position=(0, hp))
                    # evict PV unscaled right away to free the accumulator
                    a_t = p_at.tile([P, CHUNK], f32r, tag="at", name=f"at{p}_{ch}")
                    nc.vector.tensor_copy(a_t[:], pv[:])
                    if noz:
                        aT[p][ch] = a_t
                        return
                    # ---- softmax denominators -> broadcast 1/Z ----
                    rz_pair = p_rz.tile([P, CHUNK], f32, tag="rz",
                                        name=f"rz{p}_{ch}")
                    for h in range(2):
                        rz_row = p_rzrow.tile([1, CHUNK], f32, tag="rzrow",
                                              name=f"rzr{p}_{ch}_{h}")
                        for sg in range(NSEG_CH):
                            z_ps = p_aux.tile([1, SEG], f32, tag="aux",
                                              name=f"zps{p}_{ch}_{h}_{sg}")
                            nc.tensor.matmul(z_ps[:], ones_sb[:],
                                             z_acc[h][:, sg * SEG:(sg + 1) * SEG],
                                             start=True, stop=True)
                            nc.vector.reciprocal(
                                rz_row[:, sg * SEG:(sg + 1) * SEG], z_ps[:])
                        rz_dram = p_dram.tile([1, CHUNK], f32, tag="rzd",
                                              name=f"rzd{p}_{ch}_{h}")
                        nc.sync.dma_start(rz_dram[:], rz_row[:])
                        rz_bcast_ap = bass.AP(
                            tensor=rz_dram.tensor, offset=rz_dram[:].offset,
                            ap=[[0, 64]] + [list(a) for a in rz_dram[:].ap[1:]])
                        nc.gpsimd.dma_start(rz_pair[h * 64:(h + 1) * 64, :],
                                            rz_bcast_ap)
                    nc.vector.tensor_mul(a_t[:], a_t[:].bitcast(f32),
                                         rz_pair[:])
                    aT[p][ch] = a_t

                def emit_outproj(ch):
                    last = ch == NCH - 1
                    for mt in range(CHUNK // P):
                        row0 = ch * CHUNK + mt * P
                        y_ps1 = p_aux.tile([P, SEG], f32, tag="aux",
                                           name=f"yp1{ch}_{mt}")
                        # on the final chunk the pv pool is idle; use its slot to
                        # double up psum and put one eviction on the idle ScalarE
                        if last:
                            y_ps2 = p_pv.tile([P, C - SEG], f32, tag="pv",
                                              name=f"yp2{ch}_{mt}")
                        else:
                            y_ps2 = p_aux.tile([P, C - SEG], f32, tag="aux",
                                               name=f"yp2{ch}_{mt}")
                        for p in range(NPAIR):
                            lhs = aT[p][ch][:, mt * P:(mt + 1) * P]
                            nc.tensor.matmul(y_ps1[:], lhs, wo_sb[p][:, 0:SEG],
                                             start=(p == 0), stop=(p == NPAIR - 1))
                            nc.tensor.matmul(y_ps2[:], lhs, wo_sb[p][:, SEG:C],
                                             start=(p == 0), stop=(p == NPAIR - 1))
                        y_sb = p_ysb.tile([P, C], f32, tag="ysb",
                                          name=f"ysb{ch}_{mt}")
                        nc.vector.tensor_copy(y_sb[:, 0:SEG], y_ps1[:])
                        if last:
                            nc.scalar.copy(y_sb[:, SEG:C], y_ps2[:])
                        else:
                            nc.vector.tensor_copy(y_sb[:, SEG:C], y_ps2[:])
                        nc.sync.dma_start(y[row0:row0 + P, :], y_sb[:])

                # attention; pair p+1's projections emitted after attn(p, ch0)
                # so they fill PE slack while ACT grinds through attn(p)
                for p in range(NPAIR):
                    for ch in range(NCH):
                        emit_attn(p, ch)
                        if ch == 0 and p + 1 < NPAIR:
                            emit_proj(p + 1)
                        if p == NPAIR - 1:
                            emit_outproj(ch)

    nc.compile()
    return nc


def _get_nc():
    if "nc" not in _CACHE:
        _CACHE["nc"] = _build()
    return _CACHE["nc"]


def kernel(x, Wq, bq, Wk, bk, Wv, bv, Wo, bo, **_unused):
    from concourse.bass_utils import run_bass_kernel_spmd

    x = np.ascontiguousarray(np.asarray(x, dtype=np.float32))
    Wq = np.asarray(Wq, dtype=np.float32)
    Wk = np.asarray(Wk, dtype=np.float32)
    Wv = np.asarray(Wv, dtype=np.float32)
    Wo = np.asarray(Wo, dtype=np.float32)
    bq = np.asarray(bq, dtype=np.float32)
    bv = np.asarray(bv, dtype=np.float32)
    bo = np.asarray(bo, dtype=np.float32)

    ones = np.ones((P, 1), dtype=np.float32)
    in_maps = []
    for c in range(8):
        b, g = c // 2, c % 2
        sel = slice(g * DV, (g + 1) * DV)
        in_maps.append({
            "xT": np.ascontiguousarray(x[b].T),
            "wqT": np.ascontiguousarray(Wq[sel, :].T),
            "wkT": np.ascontiguousarray(Wk[sel, :].T),
            "wvT": np.ascontiguousarray(Wv[sel, :].T),
            "woT": np.ascontiguousarray(Wo[:, sel].T),
            "bq": np.ascontiguousarray(bq[sel].reshape(DV, 1)),
            "bv": np.ascontiguousarray(bv[sel].reshape(1, DV)),
            "ones": ones,
        })

    nc = _get_nc()
    res = run_bass_kernel_spmd(nc, in_maps, core_ids=list(range(8)),
                               trace=bool(_CACHE.get("trace", False)))
    _CACHE["last_result"] = res

    out = np.empty((B, N, C), dtype=np.float32)
    for b in range(B):
        out[b] = res.results[2 * b]["y"] + res.results[2 * b + 1]["y"] + bo
    return out

